# revision 1
# baseline (speedup 1.0000x reference)
"""ALIGNNConv edge-gate kernel for 8 TRN2 NeuronCores — v5 (no projection,
edge-major gathers, feature-major compute).

reference math:
    row, col = edge_index
    x = concat([nf[row], nf[col], ef], -1)        # [E, 384]
    gate = sigmoid(x @ W + b)                     # [E, 128]
    out = ef * gate

The kernel is HBM-bandwidth bound (all 8 cores share the stacks), so v5
minimizes bytes: NO precomputed projection tables. The SWDGE dma_gather
pulls raw 256-byte node-feature rows (edge-major, the only gather mode
this hardware supports), the PE transposes each gathered 128-edge subtile
into PSUM, a copy moves it to SBUF feature-major, and each 512-edge PSUM
bank accumulates three stationary-weight matmuls
    psum = W1^T @ gr_t + W2^T @ gc_t + W3^T @ ef
followed by sigmoid(psum + b) on the scalar engine (per-partition bias)
and ef*gate on the vector engine. Per-core HBM traffic is ~89MB vs ~128MB
for the projection-table variant; the extra PE/ACT/DVE work hides under
the DMA (measured: full compute adds <30us over pure streams).

dma_gather limits (probed on HW): DRAM source, transpose=False only,
num_idxs <= 1024 per call, 256B-multiple rows. int16 indices, so the node
table is used as two 25600-row halves and the host sorts each core's
edges into four sections by (row-half, col-half), row-sorted within a
section so row gathers hit HBM nearly sequentially. Sections are padded
to a fixed capacity (multiple of 512) so the program is data-independent.

Host-side work is layout-only: dtype casts (f32->bf16, int64->int16),
transposes, the bucket permutation (undone on output), and the final
bf16->f32 upcast.
"""

import os as _os

import numpy as np
import ml_dtypes

BF16 = ml_dtypes.bfloat16

N_NODES = 50000
N_EDGES = 640000
D = 128
N_CORES = 8
NODES_PAD = 51200
H = NODES_PAD // 2  # 25600 rows per table half (< 32768 for int16)
GROUP_MAX = int(_os.environ.get("V5_GROUP_MAX", "1024"))
SCRATCH = int(_os.environ.get("V5_SCRATCH", "65536"))


def _section_groups(cap):
    gs = []
    while cap >= GROUP_MAX:
        gs.append(GROUP_MAX)
        cap -= GROUP_MAX
    if cap > 0:
        gs.append(cap)
    return gs


class Cfg:
    def __init__(self, sec_cap):
        assert len(sec_cap) == 4
        for c in sec_cap:
            assert c % 512 == 0 and c > 0
        self.sec_cap = tuple(int(c) for c in sec_cap)
        self.sec_off = tuple(sum(self.sec_cap[:s]) for s in range(5))
        self.e_slots = self.sec_off[4]
        self.groups = []
        for s in range(4):
            off = self.sec_off[s]
            for g in _section_groups(self.sec_cap[s]):
                self.groups.append((s, off, g))
                off += g


E_CORE = N_EDGES // N_CORES


def build_nc(cfg: Cfg, repeat: int = 1, variant: str = "full"):
    """repeat > 1 wraps the whole body in a For_i loop for benchmarking.

    variant: benchmark-only ablations (results garbage unless "full"):
      full       - the real kernel
      nogather   - no dma_gather; logits = ef@W3 only
      nocompute  - no matmul/sigmoid/mul; out streams ef back out
      gatheronly - idx loads + gathers + out writes only
    """
    assert variant in ("full", "nogather", "nocompute", "gatheronly")
    no_gather = variant in ("nogather", "nocompute")
    no_compute = variant in ("nocompute", "gatheronly")
    no_edgedma = variant == "gatheronly"
    import concourse.bass as bass
    import concourse.mybir as mybir
    from concourse import bacc
    from concourse import library_config
    from concourse.tile import TileContext
    from concourse.tile_rust import add_dep_helper

    f32 = mybir.dt.float32
    bf16 = mybir.dt.bfloat16
    i16 = mybir.dt.int16

    nc = bacc.Bacc(
        "TRN2",
        target_bir_lowering=False,
        debug=False,
        num_swdge_queues=4,
        dynamic_dma_scratch_size=SCRATCH,
    )

    nfn = nc.declare_dram_parameter("nfn", [NODES_PAD, D], bf16, isOutput=False)
    w = nc.declare_dram_parameter("w", [3 * D, D], bf16, isOutput=False)
    bvec = nc.declare_dram_parameter("bvec", [D, 1], f32, isOutput=False)
    ident = nc.declare_dram_parameter("ident", [D, D], bf16, isOutput=False)
    n_iw = cfg.e_slots // 16
    idxr = nc.declare_dram_parameter("idxr", [D, n_iw], i16, isOutput=False)
    idxc = nc.declare_dram_parameter("idxc", [D, n_iw], i16, isOutput=False)
    eft = nc.declare_dram_parameter("eft", [D, cfg.e_slots], bf16, isOutput=False)
    outp = nc.declare_dram_parameter("out", [D, cfg.e_slots], bf16, isOutput=True)

    with TileContext(nc) as tc:
        with (
            tc.tile_pool(name="const", bufs=1) as cpool,
            tc.tile_pool(name="pps", bufs=2, space="PSUM") as pps,
            tc.tile_pool(name="tps", bufs=2, space="PSUM") as tps,
            tc.tile_pool(name="gat", bufs=6) as gpool,
            tc.tile_pool(name="trs", bufs=4) as trpool,
            tc.tile_pool(name="edg", bufs=3) as epool,
            tc.tile_pool(name="gsb", bufs=3) as gspool,
        ):
            nc.gpsimd.load_library(library_config.mlp)

            w1 = cpool.tile([D, D], bf16, name="w1")
            w2 = cpool.tile([D, D], bf16, name="w2")
            w3 = cpool.tile([D, D], bf16, name="w3")
            bia = cpool.tile([D, 1], f32, name="bia")
            idt = cpool.tile([D, D], bf16, name="idt")
            ira = cpool.tile([D, n_iw], i16, name="ira")
            ica = cpool.tile([D, n_iw], i16, name="ica")

            def emit_consts():
                nc.sync.dma_start(out=w1[:], in_=w[0:D, :])
                nc.sync.dma_start(out=w2[:], in_=w[D : 2 * D, :])
                nc.sync.dma_start(out=w3[:], in_=w[2 * D : 3 * D, :])
                nc.sync.dma_start(out=bia[:], in_=bvec[:, :])
                nc.sync.dma_start(out=idt[:], in_=ident[:, :])
                nc.sync.dma_start(out=ira[:], in_=idxr[:, :])
                nc.sync.dma_start(out=ica[:], in_=idxc[:, :])

            def emit_edges():
                prev_gather = None
                qn = 0
                for sec, off, NI in cfg.groups:
                    rh, ch = sec >> 1, sec & 1
                    i16off = off // 16
                    kmax = NI // D
                    gr = gpool.tile([D, kmax, D], bf16, name="gr")
                    gc = gpool.tile([D, kmax, D], bf16, name="gc")
                    if not no_gather:
                        g1 = nc.gpsimd.dma_gather(
                            gr[:],
                            nfn[rh * H : (rh + 1) * H, :],
                            ira[:, i16off : i16off + NI // 16],
                            NI, NI, D,
                            queue_num=qn % 4,
                            single_packet=True,
                        )
                        g2 = nc.gpsimd.dma_gather(
                            gc[:],
                            nfn[ch * H : (ch + 1) * H, :],
                            ica[:, i16off : i16off + NI // 16],
                            NI, NI, D,
                            queue_num=(qn + 1) % 4,
                            single_packet=True,
                        )
                        qn += 2
                        # keep Pool-engine issue order = program order so
                        # Tile's round-robin DMASW sem lanes stay aligned
                        # with the 4-queue cycle (sems are queue-locked)
                        if prev_gather is not None:
                            add_dep_helper(
                                g1.ins, prev_gather.ins, sync=False,
                                reason="swdge lane/queue alignment",
                            )
                        add_dep_helper(
                            g2.ins, g1.ins, sync=False,
                            reason="swdge lane/queue alignment",
                        )
                        prev_gather = g2
                    et = epool.tile([D, NI], bf16, name="et")
                    if not no_edgedma:
                        nc.sync.dma_start(out=et[:], in_=eft[:, off : off + NI])
                    o_t = epool.tile([D, NI], bf16, name="o_t")
                    if not no_compute:
                        EB = 4  # 128-edge subtiles per PSUM bank (512 edges)
                        for kb in range((kmax + EB - 1) // EB):
                            nb = min(EB, kmax - kb * EB)
                            bsl = slice(kb * EB * D, kb * EB * D + nb * D)
                            ps = pps.tile([D, nb * D], f32, name="ps")
                            if not no_gather:
                                grt = tps.tile([D, nb * D], bf16, name="grt")
                                gct = tps.tile([D, nb * D], bf16, name="gct")
                                for k4 in range(nb):
                                    k = kb * EB + k4
                                    psl = slice(k4 * D, (k4 + 1) * D)
                                    nc.tensor.transpose(
                                        grt[:, psl], gr[:, k, :], idt[:]
                                    )
                                    nc.tensor.transpose(
                                        gct[:, psl], gc[:, k, :], idt[:]
                                    )
                                grs = trpool.tile([D, nb * D], bf16, name="grs")
                                gcs = trpool.tile([D, nb * D], bf16, name="gcs")
                                nc.scalar.copy(grs[:], grt[:])
                                nc.vector.tensor_copy(gcs[:], gct[:])
                                nc.tensor.matmul(
                                    ps[:], w1[:], grs[:], start=True, stop=False
                                )
                                nc.tensor.matmul(
                                    ps[:], w2[:], gcs[:], start=False, stop=False
                                )
                            nc.tensor.matmul(
                                ps[:], w3[:], et[:, bsl],
                                start=no_gather, stop=True,
                            )
                            gate = gspool.tile([D, nb * D], bf16, name="gate")
                            nc.scalar.activation(
                                gate[:], ps[:],
                                mybir.ActivationFunctionType.Sigmoid,
                                bias=bia[:, 0:1],
                            )
                            nc.vector.tensor_mul(
                                o_t[:, bsl], et[:, bsl], gate[:]
                            )
                        nc.sync.dma_start(
                            out=outp[:, off : off + NI], in_=o_t[:]
                        )
                    elif not no_edgedma:
                        nc.sync.dma_start(out=outp[:, off : off + NI], in_=et[:])
                    else:
                        nc.sync.dma_start(
                            out=outp[:, off : off + NI],
                            in_=gr[:].rearrange("p k d -> p (k d)"),
                        )

            def emit_body():
                emit_consts()
                emit_edges()

            if repeat > 1:
                with tc.For_i(0, repeat, 1):
                    emit_body()
            else:
                emit_body()

    nc.finalize()
    return nc


def bucketize(cfg: Cfg, er32, ec32):
    """Sort edges into 4 (row-half, col-half) sections with fixed capacity."""
    sec = (er32 >= H).astype(np.int64) * 2 + (ec32 >= H)
    counts = np.bincount(sec, minlength=4)
    if np.any(counts > np.asarray(cfg.sec_cap)):
        raise ValueError(
            f"section sizes {counts} exceed capacity {cfg.sec_cap}; "
            f"node distribution too skewed for compiled bucket layout"
        )
    # sort within each section to give the gathers HBM locality:
    #   row:    row-gathers nearly sequential, col random
    #   z:      Morton order — both sides get multi-scale locality
    #   rowblk: 256-row blocks, col-sorted inside each block
    mode = _os.environ.get("V5_SORT", "z")
    if mode == "row":
        key = er32
    elif mode == "z":
        def spread(v):
            v = v.astype(np.int64)
            v = (v | (v << 8)) & 0x00FF00FF00FF00FF
            v = (v | (v << 4)) & 0x0F0F0F0F0F0F0F0F
            v = (v | (v << 2)) & 0x3333333333333333
            v = (v | (v << 1)) & 0x5555555555555555
            return v
        key = spread(er32) << 1 | spread(ec32)
    elif mode == "rowblk":
        key = (er32.astype(np.int64) >> 8) << 32 | ec32
    else:
        raise ValueError(mode)
    order = np.lexsort((key, sec))
    perm = np.full(cfg.e_slots, -1, dtype=np.int64)
    off = 0
    for s in range(4):
        n = counts[s]
        perm[cfg.sec_off[s] : cfg.sec_off[s] + n] = order[off : off + n]
        off += n
    return perm, counts


def wrap_idx(cfg: Cfg, idx_slot):
    """[e_slots] int16 -> [128, e_slots//16] dma_gather index layout."""
    parts = []
    for s, off, NI in cfg.groups:
        a = idx_slot[off : off + NI].reshape(NI // 16, 16).T  # [16, NI//16]
        parts.append(np.tile(a, (8, 1)))                      # [128, NI//16]
    return np.ascontiguousarray(np.concatenate(parts, axis=1))


def make_in_maps(cfg: Cfg, node_features, edge_index, edge_features, W, b, n_cores):
    nf = np.asarray(node_features, dtype=np.float32)
    nf_pad = np.zeros((NODES_PAD, D), dtype=np.float32)
    nf_pad[: nf.shape[0]] = nf
    nfn = np.ascontiguousarray(nf_pad.astype(BF16))

    w_bf = np.ascontiguousarray(np.asarray(W, dtype=np.float32).astype(BF16))
    bv = np.asarray(b, dtype=np.float32).reshape(D, 1)
    ident = np.eye(D, dtype=BF16)

    ei = np.asarray(edge_index)
    ef = np.asarray(edge_features, dtype=np.float32)

    e_core = ei.shape[1] // n_cores
    in_maps = []
    perms = []
    for i in range(n_cores):
        sl = slice(i * e_core, (i + 1) * e_core)
        er32 = ei[0, sl].astype(np.int32)
        ec32 = ei[1, sl].astype(np.int32)
        ef_bf = ef[sl].astype(BF16)

        perm, _ = bucketize(cfg, er32, ec32)
        filled = perm >= 0
        src = perm[filled]

        er_slot = np.zeros(cfg.e_slots, dtype=np.int32)
        ec_slot = np.zeros(cfg.e_slots, dtype=np.int32)
        er_slot[filled] = er32[src]
        ec_slot[filled] = ec32[src]
        sec_of_slot = np.repeat(np.arange(4), np.asarray(cfg.sec_cap))
        er_slot -= ((sec_of_slot >> 1) * H).astype(np.int32)
        ec_slot -= ((sec_of_slot & 1) * H).astype(np.int32)
        er_slot[~filled] = 0
        ec_slot[~filled] = 0
        assert er_slot.min() >= 0 and er_slot.max() < H
        assert ec_slot.min() >= 0 and ec_slot.max() < H

        ef_slot = np.zeros((cfg.e_slots, D), dtype=BF16)
        ef_slot[filled] = ef_bf[src]

        in_maps.append(
            {
                "nfn": nfn,
                "w": w_bf,
                "bvec": bv,
                "ident": ident,
                "idxr": wrap_idx(cfg, er_slot.astype(np.int16)),
                "idxc": wrap_idx(cfg, ec_slot.astype(np.int16)),
                "eft": np.ascontiguousarray(ef_slot.T),
            }
        )
        perms.append(perm)
    return in_maps, perms


def unpack_out(cfg: Cfg, o, perm, e_core):
    """[D, e_slots] bf16 feat-major slot output -> [e_core, D] f32."""
    slots = np.asarray(o).T.astype(np.float32)  # [e_slots, D]
    res = np.empty((e_core, D), dtype=np.float32)
    filled = perm >= 0
    res[perm[filled]] = slots[filled]
    return res


_CACHE = {}


def derive_cfg(edge_index, n_cores):
    """Pick per-section capacities from the actual index distribution."""
    ei = np.asarray(edge_index)
    e_core = ei.shape[1] // n_cores
    mx = np.zeros(4, dtype=np.int64)
    for i in range(n_cores):
        sl = slice(i * e_core, (i + 1) * e_core)
        sec = (ei[0, sl] >= H).astype(np.int64) * 2 + (ei[1, sl] >= H)
        mx = np.maximum(mx, np.bincount(sec, minlength=4))
    caps = [int(-(-int(c) // 512)) * 512 for c in mx]
    return Cfg(caps)


def kernel(node_features, edge_index, edge_features, W, b):
    from concourse.bass_utils import run_bass_kernel_spmd

    cfg = derive_cfg(edge_index, N_CORES)
    key = cfg.sec_cap
    if key not in _CACHE:
        _CACHE[key] = build_nc(cfg)
    nc = _CACHE[key]

    in_maps, perms = make_in_maps(
        cfg, node_features, edge_index, edge_features, W, b, N_CORES
    )
    res = run_bass_kernel_spmd(nc, in_maps, core_ids=list(range(N_CORES)))
    e_core = np.asarray(edge_index).shape[1] // N_CORES
    outs = [
        unpack_out(cfg, res.results[i]["out"], perms[i], e_core)
        for i in range(N_CORES)
    ]
    return np.concatenate(outs, axis=0)



# revision 13
# speedup vs baseline: 1.0482x; 1.0482x over previous
"""ALIGNNConv edge-gate kernel for 8 TRN2 NeuronCores — v5 (no projection,
edge-major gathers, feature-major compute).

reference math:
    row, col = edge_index
    x = concat([nf[row], nf[col], ef], -1)        # [E, 384]
    gate = sigmoid(x @ W + b)                     # [E, 128]
    out = ef * gate

The kernel is HBM-bandwidth bound (all 8 cores share the stacks), so v5
minimizes bytes: NO precomputed projection tables. The SWDGE dma_gather
pulls raw 256-byte node-feature rows (edge-major, the only gather mode
this hardware supports), the PE transposes each gathered 128-edge subtile
into PSUM, a copy moves it to SBUF feature-major, and each 512-edge PSUM
bank accumulates three stationary-weight matmuls
    psum = W1^T @ gr_t + W2^T @ gc_t + W3^T @ ef
followed by sigmoid(psum + b) on the scalar engine (per-partition bias)
and ef*gate on the vector engine. Per-core HBM traffic is ~89MB vs ~128MB
for the projection-table variant; the extra PE/ACT/DVE work hides under
the DMA (measured: full compute adds <30us over pure streams).

dma_gather limits (probed on HW): DRAM source, transpose=False only,
num_idxs <= 1024 per call, 256B-multiple rows. int16 indices, so the node
table is used as two 25600-row halves and the host sorts each core's
edges into four sections by (row-half, col-half), row-sorted within a
section so row gathers hit HBM nearly sequentially. Sections are padded
to a fixed capacity (multiple of 512) so the program is data-independent.

Host-side work is layout-only: dtype casts (f32->bf16, int64->int16),
transposes, the bucket permutation (undone on output), and the final
bf16->f32 upcast.
"""

import os as _os

import numpy as np
import ml_dtypes

BF16 = ml_dtypes.bfloat16

N_NODES = 50000
N_EDGES = 640000
D = 128
N_CORES = 8
NODES_PAD = 51200
H = NODES_PAD // 2  # 25600 rows per table half (< 32768 for int16)
GROUP_MAX = int(_os.environ.get("V5_GROUP_MAX", "1024"))
SCRATCH = int(_os.environ.get("V5_SCRATCH", "65536"))


def _section_groups(cap):
    gs = []
    while cap >= GROUP_MAX:
        gs.append(GROUP_MAX)
        cap -= GROUP_MAX
    if cap > 0:
        gs.append(cap)
    return gs


class Cfg:
    def __init__(self, sec_cap):
        assert len(sec_cap) == 4
        for c in sec_cap:
            assert c % 512 == 0 and c > 0
        self.sec_cap = tuple(int(c) for c in sec_cap)
        self.sec_off = tuple(sum(self.sec_cap[:s]) for s in range(5))
        self.e_slots = self.sec_off[4]
        self.groups = []
        for s in range(4):
            off = self.sec_off[s]
            for g in _section_groups(self.sec_cap[s]):
                self.groups.append((s, off, g))
                off += g


E_CORE = N_EDGES // N_CORES


def build_nc(cfg: Cfg, repeat: int = 1, variant: str = "full"):
    """repeat > 1 wraps the whole body in a For_i loop for benchmarking.

    variant: benchmark-only ablations (results garbage unless "full"):
      full       - the real kernel
      nogather   - no dma_gather; logits = ef@W3 only
      nocompute  - no matmul/sigmoid/mul; out streams ef back out
      gatheronly - idx loads + gathers + out writes only
    """
    assert variant in (
        "full", "nogather", "nocompute", "gatheronly", "gather512", "stream4k"
    )
    no_gather = variant in ("nogather", "nocompute")
    no_compute = variant in ("nocompute", "gatheronly")
    no_edgedma = variant == "gatheronly"
    import concourse.bass as bass
    import concourse.mybir as mybir
    from concourse import bacc
    from concourse import library_config
    from concourse.tile import TileContext
    from concourse.tile_rust import add_dep_helper

    f32 = mybir.dt.float32
    bf16 = mybir.dt.bfloat16
    i16 = mybir.dt.int16

    nc = bacc.Bacc(
        "TRN2",
        target_bir_lowering=False,
        debug=False,
        num_swdge_queues=4,
        dynamic_dma_scratch_size=SCRATCH,
    )

    nfn = nc.declare_dram_parameter("nfn", [NODES_PAD, D], bf16, isOutput=False)
    if variant == "gather512":
        # overlapping pair table: row n = concat(nf[n], nf[n+1]); a 512B
        # descriptor fetches two consecutive node rows
        nfp = nc.declare_dram_parameter("nfp", [NODES_PAD, 2 * D], bf16, isOutput=False)
    w = nc.declare_dram_parameter("w", [3 * D, D], bf16, isOutput=False)
    bvec = nc.declare_dram_parameter("bvec", [D, 1], f32, isOutput=False)
    ident = nc.declare_dram_parameter("ident", [D, D], bf16, isOutput=False)
    n_iw = cfg.e_slots // 16
    idxr = nc.declare_dram_parameter("idxr", [D, n_iw], i16, isOutput=False)
    idxc = nc.declare_dram_parameter("idxc", [D, n_iw], i16, isOutput=False)
    eft = nc.declare_dram_parameter("eft", [D, cfg.e_slots], bf16, isOutput=False)
    outp = nc.declare_dram_parameter("out", [D, cfg.e_slots], bf16, isOutput=True)

    with TileContext(nc) as tc:
        with (
            tc.tile_pool(name="const", bufs=1) as cpool,
            tc.tile_pool(name="pps", bufs=4, space="PSUM") as pps,
            tc.tile_pool(name="tps", bufs=2, space="PSUM") as tps,
            tc.tile_pool(name="gat", bufs=6) as gpool,
            tc.tile_pool(name="trs", bufs=4) as trpool,
            tc.tile_pool(name="edg", bufs=3) as epool,
            tc.tile_pool(name="gsb", bufs=3) as gspool,
        ):
            nc.gpsimd.load_library(library_config.mlp)

            w1 = cpool.tile([D, D], bf16, name="w1")
            w2 = cpool.tile([D, D], bf16, name="w2")
            w3 = cpool.tile([D, D], bf16, name="w3")
            bia = cpool.tile([D, 1], f32, name="bia")
            idt = cpool.tile([D, D], bf16, name="idt")
            ira = cpool.tile([D, n_iw], i16, name="ira")
            ica = cpool.tile([D, n_iw], i16, name="ica")

            def emit_consts():
                nc.sync.dma_start(out=w1[:], in_=w[0:D, :])
                nc.sync.dma_start(out=w2[:], in_=w[D : 2 * D, :])
                nc.sync.dma_start(out=w3[:], in_=w[2 * D : 3 * D, :])
                nc.sync.dma_start(out=bia[:], in_=bvec[:, :])
                nc.sync.dma_start(out=idt[:], in_=ident[:, :])
                nc.sync.dma_start(out=ira[:], in_=idxr[:, :])
                nc.sync.dma_start(out=ica[:], in_=idxc[:, :])

            def emit_stream4k():
                CH = int(_os.environ.get("V5_STREAM_CHUNK", "4096"))
                mix = _os.environ.get("V5_STREAM_ENG", "sync") == "mix"
                for off in range(0, cfg.e_slots, CH):
                    n = min(CH, cfg.e_slots - off)
                    t = epool.tile([D, n], bf16, name="st")
                    nc.sync.dma_start(out=t[:], in_=eft[:, off : off + n])
                    eng = nc.scalar if mix else nc.sync
                    eng.dma_start(out=outp[:, off : off + n], in_=t[:])

            def emit_gather512():
                prev_gather = None
                qn = 0
                for sec, off, NI in cfg.groups:
                    rh, ch = sec >> 1, sec & 1
                    i16off = off // 16
                    kmax = NI // D
                    gr = gpool.tile([D, kmax, 2 * D], bf16, name="gr")
                    gc = gpool.tile([D, kmax, 2 * D], bf16, name="gc")
                    g1 = nc.gpsimd.dma_gather(
                        gr[:],
                        nfp[rh * H : rh * H + H, :],
                        ira[:, i16off : i16off + NI // 16],
                        NI, NI, 2 * D,
                        queue_num=qn % 4,
                        single_packet=True,
                    )
                    g2 = nc.gpsimd.dma_gather(
                        gc[:],
                        nfp[ch * H : ch * H + H, :],
                        ica[:, i16off : i16off + NI // 16],
                        NI, NI, 2 * D,
                        queue_num=(qn + 1) % 4,
                        single_packet=True,
                    )
                    qn += 2
                    if prev_gather is not None:
                        add_dep_helper(
                            g1.ins, prev_gather.ins, sync=False,
                            reason="swdge lane/queue alignment",
                        )
                    add_dep_helper(
                        g2.ins, g1.ins, sync=False,
                        reason="swdge lane/queue alignment",
                    )
                    prev_gather = g2
                    nc.sync.dma_start(
                        out=outp[:, off : off + NI].rearrange(
                            "p (k d) -> p k d", k=kmax
                        ),
                        in_=gr[:, :, 0:D],
                    )

            def emit_edges():
                # ef/out stream in big chunks (independent of gather groups):
                # 1MB DMAs run ~33% faster than 256KB ones (measured)
                CH = int(_os.environ.get("V5_STREAM_CHUNK", "4096"))
                echunks = {}
                ochunks = {}

                def chunk_of(go):
                    c = go // CH
                    if c not in echunks:
                        coff = c * CH
                        n = min(CH, cfg.e_slots - coff)
                        et = epool.tile([D, n], bf16, name="etc")
                        if not no_edgedma:
                            nc.sync.dma_start(
                                out=et[:], in_=eft[:, coff : coff + n]
                            )
                        echunks[c] = (et, coff, n)
                        ochunks[c] = epool.tile([D, n], bf16, name="otc")
                    et, coff, n = echunks[c]
                    return et, ochunks[c], go - coff

                def flush_chunks(upto):
                    done = [
                        c for c, (et, coff, n) in echunks.items()
                        if coff + n <= upto
                    ]
                    for c in sorted(done):
                        et, coff, n = echunks[c]
                        if not no_compute:
                            nc.sync.dma_start(
                                out=outp[:, coff : coff + n], in_=ochunks[c]
                            )
                        elif not no_edgedma:
                            nc.sync.dma_start(
                                out=outp[:, coff : coff + n], in_=et
                            )
                        del echunks[c], ochunks[c]

                prev_gather = None
                qn = 0
                for sec, off, NI in cfg.groups:
                    rh, ch = sec >> 1, sec & 1
                    i16off = off // 16
                    kmax = NI // D
                    gr = gpool.tile([D, kmax, D], bf16, name="gr")
                    gc = gpool.tile([D, kmax, D], bf16, name="gc")
                    if not no_gather:
                        g1 = nc.gpsimd.dma_gather(
                            gr[:],
                            nfn[rh * H : (rh + 1) * H, :],
                            ira[:, i16off : i16off + NI // 16],
                            NI, NI, D,
                            queue_num=qn % 4,
                            single_packet=True,
                        )
                        g2 = nc.gpsimd.dma_gather(
                            gc[:],
                            nfn[ch * H : (ch + 1) * H, :],
                            ica[:, i16off : i16off + NI // 16],
                            NI, NI, D,
                            queue_num=(qn + 1) % 4,
                            single_packet=True,
                        )
                        qn += 2
                        # keep Pool-engine issue order = program order so
                        # Tile's round-robin DMASW sem lanes stay aligned
                        # with the 4-queue cycle (sems are queue-locked)
                        if prev_gather is not None:
                            add_dep_helper(
                                g1.ins, prev_gather.ins, sync=False,
                                reason="swdge lane/queue alignment",
                            )
                        add_dep_helper(
                            g2.ins, g1.ins, sync=False,
                            reason="swdge lane/queue alignment",
                        )
                        prev_gather = g2
                    if not no_compute:
                        EB = 4  # 128-edge subtiles per PSUM bank (512 edges)
                        for kb in range((kmax + EB - 1) // EB):
                            nb = min(EB, kmax - kb * EB)
                            bsl = slice(kb * EB * D, kb * EB * D + nb * D)
                            et, o_t, rel = chunk_of(off + kb * EB * D)
                            esl = slice(rel, rel + nb * D)
                            ps = pps.tile([D, nb * D], f32, name="ps")
                            if not no_gather:
                                grt = tps.tile([D, nb * D], bf16, name="grt")
                                gct = tps.tile([D, nb * D], bf16, name="gct")
                                for k4 in range(nb):
                                    k = kb * EB + k4
                                    psl = slice(k4 * D, (k4 + 1) * D)
                                    nc.tensor.transpose(
                                        grt[:, psl], gr[:, k, :], idt[:]
                                    )
                                    nc.tensor.transpose(
                                        gct[:, psl], gc[:, k, :], idt[:]
                                    )
                                grs = trpool.tile([D, nb * D], bf16, name="grs")
                                gcs = trpool.tile([D, nb * D], bf16, name="gcs")
                                nc.scalar.copy(grs[:], grt[:])
                                nc.vector.tensor_copy(gcs[:], gct[:])
                                nc.tensor.matmul(
                                    ps[:], w1[:], grs[:], start=True, stop=False
                                )
                                nc.tensor.matmul(
                                    ps[:], w2[:], gcs[:], start=False, stop=False
                                )
                            nc.tensor.matmul(
                                ps[:], w3[:], et[:, esl],
                                start=no_gather, stop=True,
                            )
                            gate = gspool.tile([D, nb * D], bf16, name="gate")
                            nc.scalar.activation(
                                gate[:], ps[:],
                                mybir.ActivationFunctionType.Sigmoid,
                                bias=bia[:, 0:1],
                            )
                            nc.vector.tensor_mul(
                                o_t[:, esl], et[:, esl], gate[:]
                            )
                    elif not no_edgedma:
                        chunk_of(off)
                    else:
                        nc.sync.dma_start(
                            out=outp[:, off : off + NI],
                            in_=gr[:].rearrange("p k d -> p (k d)"),
                        )
                    flush_chunks(off + NI)
                flush_chunks(cfg.e_slots + 1)

            def emit_body():
                emit_consts()
                if variant == "stream4k":
                    emit_stream4k()
                elif variant == "gather512":
                    emit_gather512()
                else:
                    emit_edges()

            if repeat > 1:
                with tc.For_i(0, repeat, 1):
                    emit_body()
            else:
                emit_body()

    nc.finalize()
    return nc


def bucketize(cfg: Cfg, er32, ec32):
    """Sort edges into 4 (row-half, col-half) sections with fixed capacity."""
    sec = (er32 >= H).astype(np.int64) * 2 + (ec32 >= H)
    counts = np.bincount(sec, minlength=4)
    if np.any(counts > np.asarray(cfg.sec_cap)):
        raise ValueError(
            f"section sizes {counts} exceed capacity {cfg.sec_cap}; "
            f"node distribution too skewed for compiled bucket layout"
        )
    # sort within each section to give the gathers HBM locality:
    #   row:    row-gathers nearly sequential, col random
    #   z:      Morton order — both sides get multi-scale locality
    #   rowblk: 256-row blocks, col-sorted inside each block
    mode = _os.environ.get("V5_SORT", "z")
    if mode == "row":
        key = er32
    elif mode == "z":
        def spread(v):
            v = v.astype(np.int64)
            v = (v | (v << 8)) & 0x00FF00FF00FF00FF
            v = (v | (v << 4)) & 0x0F0F0F0F0F0F0F0F
            v = (v | (v << 2)) & 0x3333333333333333
            v = (v | (v << 1)) & 0x5555555555555555
            return v
        key = spread(er32) << 1 | spread(ec32)
    elif mode == "rowblk":
        key = (er32.astype(np.int64) >> 8) << 32 | ec32
    else:
        raise ValueError(mode)
    order = np.lexsort((key, sec))
    perm = np.full(cfg.e_slots, -1, dtype=np.int64)
    off = 0
    for s in range(4):
        n = counts[s]
        perm[cfg.sec_off[s] : cfg.sec_off[s] + n] = order[off : off + n]
        off += n
    return perm, counts


def wrap_idx(cfg: Cfg, idx_slot):
    """[e_slots] int16 -> [128, e_slots//16] dma_gather index layout."""
    parts = []
    for s, off, NI in cfg.groups:
        a = idx_slot[off : off + NI].reshape(NI // 16, 16).T  # [16, NI//16]
        parts.append(np.tile(a, (8, 1)))                      # [128, NI//16]
    return np.ascontiguousarray(np.concatenate(parts, axis=1))


def make_in_maps(cfg: Cfg, node_features, edge_index, edge_features, W, b, n_cores):
    nf = np.asarray(node_features, dtype=np.float32)
    nf_pad = np.zeros((NODES_PAD, D), dtype=np.float32)
    nf_pad[: nf.shape[0]] = nf
    nfn = np.ascontiguousarray(nf_pad.astype(BF16))
    nfp = np.zeros((NODES_PAD, 2 * D), dtype=BF16)
    nfp[:, :D] = nfn
    nfp[:-1, D:] = nfn[1:]

    w_bf = np.ascontiguousarray(np.asarray(W, dtype=np.float32).astype(BF16))
    bv = np.asarray(b, dtype=np.float32).reshape(D, 1)
    ident = np.eye(D, dtype=BF16)

    ei = np.asarray(edge_index)
    ef = np.asarray(edge_features, dtype=np.float32)

    e_core = ei.shape[1] // n_cores
    in_maps = []
    perms = []
    for i in range(n_cores):
        sl = slice(i * e_core, (i + 1) * e_core)
        er32 = ei[0, sl].astype(np.int32)
        ec32 = ei[1, sl].astype(np.int32)
        ef_bf = ef[sl].astype(BF16)

        perm, _ = bucketize(cfg, er32, ec32)
        filled = perm >= 0
        src = perm[filled]

        er_slot = np.zeros(cfg.e_slots, dtype=np.int32)
        ec_slot = np.zeros(cfg.e_slots, dtype=np.int32)
        er_slot[filled] = er32[src]
        ec_slot[filled] = ec32[src]
        sec_of_slot = np.repeat(np.arange(4), np.asarray(cfg.sec_cap))
        er_slot -= ((sec_of_slot >> 1) * H).astype(np.int32)
        ec_slot -= ((sec_of_slot & 1) * H).astype(np.int32)
        er_slot[~filled] = 0
        ec_slot[~filled] = 0
        assert er_slot.min() >= 0 and er_slot.max() < H
        assert ec_slot.min() >= 0 and ec_slot.max() < H

        ef_slot = np.zeros((cfg.e_slots, D), dtype=BF16)
        ef_slot[filled] = ef_bf[src]

        in_maps.append(
            {
                "nfn": nfn,
                "nfp": nfp,
                "w": w_bf,
                "bvec": bv,
                "ident": ident,
                "idxr": wrap_idx(cfg, er_slot.astype(np.int16)),
                "idxc": wrap_idx(cfg, ec_slot.astype(np.int16)),
                "eft": np.ascontiguousarray(ef_slot.T),
            }
        )
        perms.append(perm)
    return in_maps, perms


def unpack_out(cfg: Cfg, o, perm, e_core):
    """[D, e_slots] bf16 feat-major slot output -> [e_core, D] f32."""
    slots = np.asarray(o).T.astype(np.float32)  # [e_slots, D]
    res = np.empty((e_core, D), dtype=np.float32)
    filled = perm >= 0
    res[perm[filled]] = slots[filled]
    return res


_CACHE = {}


def derive_cfg(edge_index, n_cores):
    """Pick per-section capacities from the actual index distribution."""
    ei = np.asarray(edge_index)
    e_core = ei.shape[1] // n_cores
    mx = np.zeros(4, dtype=np.int64)
    for i in range(n_cores):
        sl = slice(i * e_core, (i + 1) * e_core)
        sec = (ei[0, sl] >= H).astype(np.int64) * 2 + (ei[1, sl] >= H)
        mx = np.maximum(mx, np.bincount(sec, minlength=4))
    caps = [int(-(-int(c) // 512)) * 512 for c in mx]
    return Cfg(caps)


def kernel(node_features, edge_index, edge_features, W, b):
    from concourse.bass_utils import run_bass_kernel_spmd

    cfg = derive_cfg(edge_index, N_CORES)
    key = cfg.sec_cap
    if key not in _CACHE:
        _CACHE[key] = build_nc(cfg)
    nc = _CACHE[key]

    in_maps, perms = make_in_maps(
        cfg, node_features, edge_index, edge_features, W, b, N_CORES
    )
    res = run_bass_kernel_spmd(nc, in_maps, core_ids=list(range(N_CORES)))
    e_core = np.asarray(edge_index).shape[1] // N_CORES
    outs = [
        unpack_out(cfg, res.results[i]["out"], perms[i], e_core)
        for i in range(N_CORES)
    ]
    return np.concatenate(outs, axis=0)

